# revision 13
# baseline (speedup 1.0000x reference)
"""Trainium2 Bass kernel for a channel-attention block.

Per batch b (one NeuronCore each, 8 total):
    v      = x[b].reshape(C, H*W)                    # [256, 16384]
    energy = v @ v.T                                 # [256, 256]
    w      = softmax(max(energy, -1) - energy, -1)   # == softmax(-energy)
    y      = alpha * (w @ v) + x[b]

Layout / strategy (per core):
  - v stays resident in SBUF as [128, 2, 16384] f32r (c = h*128 + p).
  - Phase B: each 128-wide s-tile of v is PE-transposed and fed to two
    f32r matmuls (FP22, bf16-rate at N>=256) accumulating [128, 256]
    PSUM tiles.  On HW the PE *sequencer* (~65 ns/instr, 8 PE instrs
    per k-tile) is the phase-B pacer, so the PE starts as early as
    possible (first input piece is 512 KB) and transposes+copies run
    three pairs ahead of the matmuls.  All PSUM tiles are padded to a
    full 2 KB bank to avoid bank-sharing hazards.
  - Phase C: stable softmax via reduce-min + fused exp(min - e) with
    accumulated row-sum.  1/sum (and alpha) are NOT applied to w; they
    fold into phase D's scalar_tensor_tensor as a per-partition scalar,
    removing the normalize multiplies from the critical path.  wT
    copies split across DVE/ACT.
  - Phase D: y = rc*alpha*(wHat @ v) + x fused on VectorE; output
    staged in SBUF and streamed with 512 KB head/tail pieces (the
    first DMA starts ~2 us after softmax) and 2 MB steady chunks.
"""

from contextlib import ExitStack

import numpy as np

import concourse.bass as bass
import concourse.mybir as mybir
import concourse.tile as tile
from concourse import bacc
from concourse.bass_utils import run_bass_kernel_spmd
from concourse.masks import make_identity

B, C, HH, WW = 8, 256, 128, 128
HW = HH * WW            # 16384
P = 128
H = C // P              # 2 channel chunks
KT = HW // P            # 128 contraction tiles for energy
S_TILE = 512            # second-matmul moving free dim (1 PSUM bank)

F32 = mybir.dt.float32
F32R = mybir.dt.float32r

# Input DMA pieces in k-tiles: small head pieces so the PE can start
# transposing after ~512 KB, then 2 MB steady chunks.
IN_PIECES = [(0, 4), (4, 8), (8, 12), (12, 16)] + [
    (t, t + 16) for t in range(16, KT, 16)
]
# Output DMA piece widths in columns: 512 KB head x4 (early DMA start),
# graded up to 2 MB steady, 512 KB tail x4 (small kernel-gating final
# DMA).  Grading keeps each piece's staging barrier under its DMA time.
OUT_PIECES = [512] * 4 + [1024] * 2 + [2048] * 5 + [512] * 4
assert sum(OUT_PIECES) == HW


def emit(nc, tc, alpha, ident_r, v_sb, x_v, y_v):
    """One full per-core pass (phases A-D). Pools are scoped inside."""
    # ---- Phase A: stream x into SBUF.
    for t0, t1 in IN_PIECES:
        sl = slice(t0 * P, t1 * P)
        nc.sync.dma_start(out=v_sb[:, :, sl], in_=x_v[:, :, sl])

    with ExitStack() as wctx:
        w_pool = wctx.enter_context(tc.tile_pool(name="w", bufs=1))
        w_sb = [w_pool.tile([P, C], F32R, name=f"w{h}") for h in range(H)]
        wt_sb = [w_pool.tile([P, C], F32R, name=f"wt{g}") for g in range(H)]
        stats = wctx.enter_context(tc.tile_pool(name="stats", bufs=1))
        rca = [stats.tile([P, 1], F32, name=f"rca{h}") for h in range(H)]

        with ExitStack() as bctx:
            vt_pool = bctx.enter_context(tc.tile_pool(name="vt", bufs=8))
            psum_e = bctx.enter_context(
                tc.tile_pool(name="psum_e", bufs=1, space="PSUM"))
            psum_t = bctx.enter_context(
                tc.tile_pool(name="psum_t", bufs=6, space="PSUM"))

            # ---- Phase B: energy = v @ v.T (two PSUM banks), transposes
            # and copies pipelined ahead of the matmuls.
            e_ps = [psum_e.tile([P, 2 * C], F32, name=f"energy{h}")[:, 0:C]
                    for h in range(H)]

            def make_vt(k):
                ksl = slice(k * P, (k + 1) * P)
                vt = vt_pool.tile([P, C], F32R, name="vt")
                tp = psum_t.tile([P, 2 * C], F32R, name="tp")[:, 0:C]
                for h in range(H):
                    nc.tensor.transpose(
                        tp[:, h * P:(h + 1) * P], v_sb[:, h, ksl], ident_r[:]
                    )
                if k % 2 == 0:
                    nc.scalar.copy(vt[:], tp)
                else:
                    nc.vector.tensor_copy(vt[:], tp)
                return vt

            # Two k-tiles per pipeline step; transposes+copies run three
            # pairs ahead so the copy latency chain never stalls the PE.
            vts = [make_vt(0), make_vt(1), make_vt(2), make_vt(3)]
            for k0 in range(0, KT, 2):
                for kn in (k0 + 4, k0 + 5):
                    if kn < KT:
                        vts.append(make_vt(kn))
                for k in (k0, k0 + 1):
                    vt_r = vts.pop(0)[:]
                    for h in range(H):
                        nc.tensor.matmul(
                            e_ps[h],
                            lhsT=vt_r[:, h * P:(h + 1) * P],
                            rhs=vt_r,
                            start=(k == 0),
                            stop=(k == KT - 1),
                        )

            # ---- Phase C: softmax(max - e) == exp(min - e)/sum.  Only the
            # exp is materialized; 1/sum * alpha folds into phase D.
            for h in range(H):
                mn = stats.tile([P, 1], F32, name=f"mn{h}")
                sm = stats.tile([P, 1], F32, name=f"sm{h}")
                rc = stats.tile([P, 1], F32, name=f"rc{h}")
                nc.vector.tensor_reduce(
                    mn[:], e_ps[h], axis=mybir.AxisListType.X,
                    op=mybir.AluOpType.min
                )
                nc.scalar.activation(
                    w_sb[h][:], e_ps[h], mybir.ActivationFunctionType.Exp,
                    bias=mn[:], scale=-1.0, accum_out=sm[:],
                )
                nc.vector.reciprocal(rc[:], sm[:])
                nc.vector.tensor_scalar_mul(rca[h][:], rc[:], float(alpha))
            # wT[g][p, h*128+q] = w[h][q, g*128+p] for the second matmul.
            # h=1 is last off the softmax chain: its copies go on the
            # faster DVE, h=0's on ACT.
            for g in range(H):
                for h in range(H):
                    tp2 = psum_t.tile([P, 2 * C], F32R, name="tp2", tag="tp")
                    nc.tensor.transpose(
                        tp2[:, 0:P], w_sb[h][:, g * P:(g + 1) * P], ident_r[:]
                    )
                    if h == 1:
                        nc.vector.tensor_copy(
                            wt_sb[g][:, h * P:(h + 1) * P], tp2[:, 0:P])
                    else:
                        nc.scalar.copy(
                            wt_sb[g][:, h * P:(h + 1) * P], tp2[:, 0:P])

        # ---- Phase D: y = rc*alpha*(wHat @ v) + v, streamed out.
        with ExitStack() as dctx:
            out_s = dctx.enter_context(tc.tile_pool(name="out_s", bufs=3))
            out_m = dctx.enter_context(tc.tile_pool(name="out_m", bufs=1))
            out_b = dctx.enter_context(tc.tile_pool(name="out_b", bufs=2))
            psum_y = dctx.enter_context(
                tc.tile_pool(name="psum_y", bufs=4, space="PSUM"))
            col = 0
            for width in OUT_PIECES:
                pool = {512: out_s, 1024: out_m, 2048: out_b}[width]
                ost = pool.tile([P, H, width], F32, name="ost")
                for m in range(H):
                    for jj in range(width // S_TILE):
                        j0 = col + jj * S_TILE
                        jsl = slice(j0, j0 + S_TILE)
                        yp = psum_y.tile([P, S_TILE], F32, name="yp")
                        for g in range(H):
                            nc.tensor.matmul(
                                yp[:],
                                lhsT=wt_sb[g][:][:, m * P:(m + 1) * P],
                                rhs=v_sb[:][:, g, jsl],
                                start=(g == 0),
                                stop=(g == H - 1),
                            )
                        nc.vector.scalar_tensor_tensor(
                            out=ost[:, m, jj * S_TILE:(jj + 1) * S_TILE],
                            in0=yp[:],
                            scalar=rca[m][:],
                            in1=v_sb[:, m, jsl].bitcast(F32),
                            op0=mybir.AluOpType.mult,
                            op1=mybir.AluOpType.add,
                        )
                osl = slice(col, col + width)
                nc.sync.dma_start(out=y_v[:, :, osl], in_=ost[:])
                col += width


def _build(alpha: float) -> bass.Bass:
    # Bacc (not plain Bass): its compile() legalizes semaphore waits into
    # EventSemaphore instructions — hardware allows only 1 wait per
    # instruction and Tile freely emits more.
    nc = bacc.Bacc("TRN2", target_bir_lowering=False)
    # x declared f32r (same 32-bit layout as fp32 at rest) so the DMA, the
    # PE transposes, and both matmuls form a consistent f32r chain.
    x = nc.dram_tensor("x", [C, HW], F32R, kind="ExternalInput")
    y = nc.dram_tensor("y", [C, HW], F32, kind="ExternalOutput")
    x_v = x.rearrange("(h p) s -> p h s", p=P)
    y_v = y.rearrange("(h p) s -> p h s", p=P)

    with tile.TileContext(nc) as tc, ExitStack() as ctx:
        singles = ctx.enter_context(tc.tile_pool(name="singles", bufs=1))
        ident = singles.tile([P, P], F32, name="ident")
        make_identity(nc, ident)
        ident_r = singles.tile([P, P], F32R, name="ident_r")
        nc.vector.tensor_copy(ident_r[:], ident[:])
        # Whole v resident: 128 KB per partition.
        v_sb = singles.tile([P, H, HW], F32R, name="v_sb")
        emit(nc, tc, alpha, ident_r, v_sb, x_v, y_v)
    nc.compile()
    return nc


def kernel(x: np.ndarray, alpha: np.ndarray, **_kw) -> np.ndarray:
    assert x.shape == (B, C, HH, WW) and x.dtype == np.float32
    xs = np.ascontiguousarray(x.reshape(B, C, HW)).astype(np.float32, copy=False)
    nc = _build(float(np.asarray(alpha).reshape(-1)[0]))
    in_maps = [{"x": xs[b]} for b in range(B)]
    res = run_bass_kernel_spmd(nc, in_maps, core_ids=list(range(B)))
    out = np.stack([np.asarray(r["y"]) for r in res.results])
    return out.reshape(B, C, HH, WW).astype(np.float32, copy=False)
